# revision 20
# baseline (speedup 1.0000x reference)
"""Trainium2 Bass kernel for nn_NeuralAttention (cross-attention with RoPE).

Sharding: 8 cores = 4 batches (data parallel) x 2 head-groups (tensor
parallel, 8 heads each).  AllReduce over TP pairs after the output
projection.

Per-core device program (SPMD, per-core data):
  phase 1: Q/K/V projections in transposed layout (channels on partitions),
           RoPE applied via a block rotate-half permutation matmul + DVE
           combine with gathered cos/sin tables (dma_gather transpose).
  phase 2: per head-pair: row-packed score matmuls (d=64 contraction),
           Exp on ScalarE with fused 1/8 scale + per-key mask bias,
           attn@V matmuls with an appended ones column producing the
           softmax denominator for free, then normalization.
  phase 3: output projection (K=64 per head) + bo/2, DMA to DRAM.
  phase 4: AllReduce over {2b, 2b+1} pairs.
"""

import numpy as np
import ml_dtypes

import concourse.bass as bass
import concourse.mybir as mybir
from concourse import bacc
import concourse.tile as tile
from concourse import library_config
from concourse.bass_utils import run_bass_kernel_spmd

B, L, T = 4, 512, 4096
HID, NH, HD = 1024, 16, 64
MAX_POS, BASE = 4096, 10000.0
G = 2                 # TP head groups
NHG = NH // G         # heads per group
C = NHG * HD          # channels per group = 512
NCORES = 8

F32 = mybir.dt.float32
F32R = mybir.dt.float32r
BF16 = mybir.dt.bfloat16
I16 = mybir.dt.int16
U8 = mybir.dt.uint8

MULT = None  # set after import
ADD = None

_BF = ml_dtypes.bfloat16


# ---------------------------------------------------------------- host prep
def _host_tables():
    inv_freq = 1.0 / BASE ** (np.arange(0, HD, 2, dtype=np.float32) / HD)
    t = np.arange(MAX_POS, dtype=np.float32)
    freqs = np.einsum('i,j->ij', t, inv_freq).astype(np.float32)
    emb = np.concatenate([freqs, freqs], axis=-1)          # [MAX_POS, HD]
    return np.cos(emb).astype(np.float32), np.sin(emb).astype(np.float32)


def _rot_perm2():
    # P: rotate_half as a linear map; P2 = blockdiag(P, P)  [128, 128]
    P = np.zeros((HD, HD), np.float32)
    for d in range(HD // 2):
        P[d, d + HD // 2] = -1.0
        P[d + HD // 2, d] = 1.0
    P2 = np.zeros((128, 128), np.float32)
    P2[:64, :64] = P
    P2[64:, 64:] = P
    return P2


# ---------------------------------------------------------------- bass build
_NC_CACHE = {}
OPTS = {"no_cc": False, "no_gather": False}


def _build_nc():
    global MULT, ADD
    MULT = mybir.AluOpType.mult
    ADD = mybir.AluOpType.add
    EXP = mybir.ActivationFunctionType.Exp

    nc = bacc.Bacc(None, target_bir_lowering=False)

    # -------- DRAM parameters (per-core data fed via in_maps)
    tgtT = nc.declare_dram_parameter("tgtT", [HID, T], BF16, isOutput=False)       # target[b].T
    latT = nc.declare_dram_parameter("latT", [HID, L], BF16, isOutput=False)       # latents[b].T
    wkT = nc.declare_dram_parameter("wkT", [HID, C], BF16, isOutput=False)         # Wk_g.T
    wvT = nc.declare_dram_parameter("wvT", [HID, C], BF16, isOutput=False)
    wqT = nc.declare_dram_parameter("wqT", [HID, C], BF16, isOutput=False)
    woT = nc.declare_dram_parameter("woT", [C, HID], BF16, isOutput=False)         # Wo.T rows of group
    pt2 = nc.declare_dram_parameter("pt2", [128, 128], BF16, isOutput=False)       # P2.T
    cosq = nc.declare_dram_parameter("cosq", [128, L], BF16, isOutput=False)       # replicated x2
    sinq = nc.declare_dram_parameter("sinq", [128, L], BF16, isOutput=False)
    tabcs = nc.declare_dram_parameter("tabcs", [MAX_POS, 256], BF16, isOutput=False)  # [cos|cos|sin|sin]
    idx32 = nc.declare_dram_parameter("idx32", [128, T // 128], mybir.dt.int32, isOutput=False)
    eye = nc.declare_dram_parameter("eye", [128, 128], BF16, isOutput=False)
    if OPTS["no_gather"]:
        coskg = nc.declare_dram_parameter("coskg", [128, T], BF16, isOutput=False)
        sinkg = nc.declare_dram_parameter("sinkg", [128, T], BF16, isOutput=False)
    maskw = nc.declare_dram_parameter("maskw", [128, T // 128], U8, isOutput=False)
    bkw = nc.declare_dram_parameter("bkw", [128, C // 128], F32, isOutput=False)
    bqw = nc.declare_dram_parameter("bqw", [128, C // 128], F32, isOutput=False)
    bvrep = nc.declare_dram_parameter("bvrep", [128, C], F32, isOutput=False)
    borep = nc.declare_dram_parameter("borep", [128, HID], F32, isOutput=False)   # bo/2 replicated

    out = nc.declare_dram_parameter("out", [L, HID], F32, isOutput=True)
    cc_in = nc.dram_tensor("cc_in", [L, HID], F32)
    cc_out = nc.dram_tensor("cc_out", [L, HID], F32)

    TS = 256                    # t-slice width for phase 1
    NSL = T // TS               # 16 slices

    def mmr(out_ap, lhsT, rhs, **kw):
        nc.tensor.matmul(out_ap, lhsT, rhs, **kw)

    with tile.TileContext(nc) as tc:
        with tc.tile_pool(name="persist", bufs=1) as persist:
            # persistent across phases
            kpr = [persist.tile([128, T], BF16, tag=f"kpr{i}", name=f"kpr{i}")
                   for i in range(4)]
            qpr = [persist.tile([128, L], BF16, tag=f"qpr{i}", name=f"qpr{i}")
                   for i in range(4)]
            v_sb = persist.tile([128, T // 128, NHG, HD + 1], BF16, tag="v_sb")
            hT = persist.tile([64, NHG, L], BF16, tag="hT")
            ones_sb = persist.tile([128, 64], F32, tag="ones")
            mb_sb = persist.tile([128, T // 128], F32, tag="mb")

            nc.vector.memset(ones_sb[64:65, :], 1.0)
            # ones column of v (per head)
            nc.vector.memset(v_sb[:, :, :, HD:HD + 1], 1.0)

            # mask -> additive bias ( (m-1)*30000 : 0 keep, -30000 drop )
            with tc.tile_pool(name="mprep", bufs=1) as mprep:
                mk_sb = mprep.tile([128, T // 128], U8, tag="mk")
                nc.sync.dma_start(out=mk_sb, in_=maskw[:, :])
                nc.vector.tensor_copy(out=mb_sb, in_=mk_sb)       # u8 -> f32
                nc.vector.tensor_scalar_add(mb_sb, mb_sb, -1.0)
                nc.vector.tensor_scalar_mul(mb_sb, mb_sb, 30000.0)

            pt2_sb = persist.tile([128, 128], BF16, tag="pt2")
            nc.sync.dma_start(out=pt2_sb, in_=pt2[:, :])

            # ======================================================= phase 1
            with tc.tile_pool(name="tgtp", bufs=2) as tgtp, \
                 tc.tile_pool(name="scr", bufs=3) as scr, \
                 tc.tile_pool(name="kps", bufs=2, space="PSUM") as kps, \
                 tc.tile_pool(name="vps", bufs=2, space="PSUM") as vps, \
                 tc.tile_pool(name="rps", bufs=2, space="PSUM") as rps:

                # ---- Q projection + rope (scoped: frees wq/lat right after)
                with tc.tile_pool(name="qc", bufs=1) as qc:
                    wq_sb = qc.tile([128, 8, C], BF16, tag="wq")
                    nc.sync.dma_start(out=wq_sb, in_=wqT[:, :].rearrange("(k p) c -> p k c", p=128))
                    lat_sb = qc.tile([128, 8, L], BF16, tag="lat")
                    nc.sync.dma_start(out=lat_sb, in_=latT[:, :].rearrange("(k p) l -> p k l", p=128))
                    bq_sb = qc.tile([128, C // 128], F32, tag="bq")
                    nc.sync.dma_start(out=bq_sb, in_=bqw[:, :])
                    cq_sb = qc.tile([128, L], BF16, tag="cq")
                    nc.sync.dma_start(out=cq_sb, in_=cosq[:, :])
                    sq_sb = qc.tile([128, L], BF16, tag="sq")
                    nc.sync.dma_start(out=sq_sb, in_=sinq[:, :])

                    for ct in range(4):
                        qp = kps.tile([128, L], F32, tag="kp")
                        for k in range(8):
                            mmr(qp, wq_sb[:, k, ct * 128:(ct + 1) * 128],
                                lat_sb[:, k, :], start=(k == 0), stop=(k == 7))
                        qsb = scr.tile([128, L], BF16, tag="ksb")
                        nc.vector.tensor_scalar_add(qsb, qp, bq_sb[:, ct:ct + 1])
                        qr = rps.tile([128, L], F32, tag="rp")
                        mmr(qr, pt2_sb, qsb, start=True, stop=True)
                        t1 = scr.tile([128, L], BF16, tag="t1")
                        nc.vector.tensor_tensor(t1, qsb, cq_sb, MULT)
                        t2 = scr.tile([128, L], BF16, tag="t2")
                        nc.vector.tensor_tensor(t2, qr, sq_sb, MULT)
                        nc.vector.tensor_tensor(qpr[ct], t1, t2, ADD)

                # ---- K/V weights + rope tables (after Q scope frees space)
                ph1c_cm = tc.tile_pool(name="ph1c", bufs=1)
                ph1c = ph1c_cm.__enter__()
                wk_sb = ph1c.tile([128, 8, C], BF16, tag="wk")
                nc.sync.dma_start(out=wk_sb, in_=wkT[:, :].rearrange("(k p) c -> p k c", p=128))
                wv_sb = ph1c.tile([128, 8, C], BF16, tag="wv")
                nc.sync.dma_start(out=wv_sb, in_=wvT[:, :].rearrange("(k p) c -> p k c", p=128))
                bk_sb = ph1c.tile([128, C // 128], F32, tag="bk")
                nc.sync.dma_start(out=bk_sb, in_=bkw[:, :])
                bv_sb = ph1c.tile([128, C], F32, tag="bv")
                nc.sync.dma_start(out=bv_sb, in_=bvrep[:, :])

                # gathered + replicated rope tables (bf16), [128, T]
                cosk_sb = ph1c.tile([128, 1, T], BF16, tag="cosk")
                sink_sb = ph1c.tile([128, 1, T], BF16, tag="sink")
                if OPTS["no_gather"]:
                    nc.sync.dma_start(out=cosk_sb[:, 0, :], in_=coskg[:, :])
                    nc.sync.dma_start(out=sink_sb[:, 0, :], in_=sinkg[:, :])
                else:
                    # gather [cos|cos|sin|sin] rows by timestamp (one row per
                    # partition per call), then PE-transpose into [chan, t]
                    idx_sb = ph1c.tile([128, T // 128], mybir.dt.int32, tag="idx")
                    nc.sync.dma_start(out=idx_sb, in_=idx32[:, :])
                    eye_sb = ph1c.tile([128, 128], BF16, tag="eye")
                    nc.sync.dma_start(out=eye_sb, in_=eye[:, :])
                    tcs_sb = ph1c.tile([128, T // 128, 256], BF16, tag="tcs")
                    with tc.tile_pool(name="tps", bufs=2, space="PSUM") as tps:
                        for tt in range(T // 128):
                            nc.gpsimd.indirect_dma_start(
                                out=tcs_sb[:, tt, :], out_offset=None,
                                in_=tabcs[:, :],
                                in_offset=bass.IndirectOffsetOnAxis(
                                    ap=idx_sb[:, tt:tt + 1], axis=0))
                        for tt in range(T // 128):
                            tpc = tps.tile([128, 128], BF16, tag="tp")
                            nc.tensor.transpose(out=tpc, in_=tcs_sb[:, tt, 0:128],
                                                identity=eye_sb)
                            nc.vector.tensor_copy(
                                out=cosk_sb[:, 0, tt * 128:(tt + 1) * 128], in_=tpc)
                            tpsn = tps.tile([128, 128], BF16, tag="tp")
                            nc.tensor.transpose(out=tpsn, in_=tcs_sb[:, tt, 128:256],
                                                identity=eye_sb)
                            nc.vector.tensor_copy(
                                out=sink_sb[:, 0, tt * 128:(tt + 1) * 128], in_=tpsn)

                # ---- K/V projections + K rope, streamed over t-slices
                for s in range(NSL):
                    tg = tgtp.tile([128, 8, TS], BF16, tag="tgt")
                    nc.sync.dma_start(
                        out=tg,
                        in_=tgtT[:, s * TS:(s + 1) * TS].rearrange("(k p) t -> p k t", p=128))
                    cosf = cosk_sb[:, 0, s * TS:(s + 1) * TS]
                    sinf = sink_sb[:, 0, s * TS:(s + 1) * TS]

                    ksbs = []
                    for ct in range(4):
                        kp = kps.tile([128, TS], F32, tag="kp")
                        for k in range(8):
                            mmr(kp, wk_sb[:, k, ct * 128:(ct + 1) * 128],
                                tg[:, k, :], start=(k == 0), stop=(k == 7))
                        ksb = scr.tile([128, TS], BF16, tag="ksb")
                        nc.vector.tensor_scalar_add(ksb, kp, bk_sb[:, ct:ct + 1])
                        ksbs.append(ksb)
                    # V while rope matmuls depend on the copies above
                    for tt in range(TS // 128):
                        vp = vps.tile([128, C], F32, tag="vp")
                        for k in range(8):
                            mmr(vp, tg[:, k, tt * 128:(tt + 1) * 128],
                                wv_sb[:, k, :], start=(k == 0), stop=(k == 7))
                        ti = s * (TS // 128) + tt
                        nc.vector.tensor_tensor(
                            v_sb[:, ti, :, 0:HD],
                            vp.rearrange("p (h d) -> p h d", h=NHG),
                            bv_sb.rearrange("p (h d) -> p h d", h=NHG), ADD)
                    for ct in range(4):
                        kr = rps.tile([128, TS], F32, tag="rp")
                        mmr(kr, pt2_sb, ksbs[ct], start=True, stop=True)
                        t1 = scr.tile([128, TS], BF16, tag="t1")
                        nc.vector.tensor_tensor(t1, ksbs[ct], cosf, MULT)
                        t2 = scr.tile([128, TS], BF16, tag="t2")
                        nc.vector.tensor_tensor(t2, kr, sinf, MULT)
                        nc.vector.tensor_tensor(
                            kpr[ct][:, s * TS:(s + 1) * TS], t1, t2, ADD)
                ph1c_cm.__exit__(None, None, None)

            # ======================================================= phase 2
            with tc.tile_pool(name="ph2c", bufs=1) as ph2c, \
                 tc.tile_pool(name="escr", bufs=4) as escr, \
                 tc.tile_pool(name="scr2", bufs=2) as scr2:

                wo_sb = ph2c.tile([64, NHG, HID], BF16, tag="wo")
                nc.sync.dma_start(out=wo_sb, in_=woT[:, :].rearrange("(h p) o -> p h o", p=64))
                bo_sb = ph2c.tile([128, HID], F32, tag="bo")
                nc.sync.dma_start(out=bo_sb, in_=borep[:, :])

                ph2p_cm = tc.tile_pool(name="sps", bufs=2, space="PSUM")
                sps = ph2p_cm.__enter__()
                avp_cm = tc.tile_pool(name="avp", bufs=1, space="PSUM")
                avp = avp_cm.__enter__()
                bcp_cm = tc.tile_pool(name="bcp", bufs=2, space="PSUM")
                bcp = bcp_cm.__enter__()

                NT = T // 128     # 32 key tiles
                for p in range(4):
                    hA, hB = 2 * p, 2 * p + 1
                    avA = avp.tile([65, L], F32, tag="avA")
                    avB = avp.tile([65, L], F32, tag="avB")
                    es = {}
                    for tt in range(NT):
                        sA = sps.tile([128, L], F32, tag="sA")
                        nc.tensor.matmul(sA, kpr[p][0:64, tt * 128:(tt + 1) * 128],
                                         qpr[p][0:64, :], start=True, stop=True)
                        sB = sps.tile([128, L], F32, tag="sB")
                        nc.tensor.matmul(sB, kpr[p][64:128, tt * 128:(tt + 1) * 128],
                                         qpr[p][64:128, :], start=True, stop=True)
                        eA = escr.tile([128, L], BF16, tag="eA")
                        nc.scalar.activation(out=eA, in_=sA, func=EXP,
                                             bias=mb_sb[:, tt:tt + 1], scale=0.125)
                        eB = escr.tile([128, L], BF16, tag="eB")
                        nc.scalar.activation(out=eB, in_=sB, func=EXP,
                                             bias=mb_sb[:, tt:tt + 1], scale=0.125)
                        es[tt] = (eA, eB)
                        # software-pipeline: issue previous tile's AV matmuls
                        if tt > 0:
                            pA, pB = es.pop(tt - 1)
                            nc.tensor.matmul(avA, v_sb[:, tt - 1, hA, :], pA,
                                             start=(tt - 1 == 0), stop=False)
                            nc.tensor.matmul(avB, v_sb[:, tt - 1, hB, :], pB,
                                             start=(tt - 1 == 0), stop=False)
                    pA, pB = es.pop(NT - 1)
                    nc.tensor.matmul(avA, v_sb[:, NT - 1, hA, :], pA,
                                     start=False, stop=True)
                    nc.tensor.matmul(avB, v_sb[:, NT - 1, hB, :], pB,
                                     start=False, stop=True)

                    for av, h in ((avA, hA), (avB, hB)):
                        dn = scr2.tile([128, L], F32, tag="dn")
                        nc.vector.tensor_copy(out=dn[64:65, :], in_=av[64:65, :])
                        nc.vector.reciprocal(out=dn[64:65, :], in_=dn[64:65, :])
                        bc = bcp.tile([64, L], F32, tag="bc")
                        nc.tensor.matmul(bc, ones_sb[64:65, :], dn[64:65, :], start=True, stop=True)
                        osb = scr2.tile([64, L], F32, tag="osb")
                        nc.vector.tensor_copy(out=osb, in_=av[0:64, :])
                        nc.vector.tensor_tensor(hT[:, h, :], osb, bc, MULT)

                bcp_cm.__exit__(None, None, None)
                avp_cm.__exit__(None, None, None)
                ph2p_cm.__exit__(None, None, None)

                # =================================================== phase 3
                with tc.tile_pool(name="ops", bufs=2, space="PSUM") as ops, \
                     tc.tile_pool(name="ow", bufs=3) as ow:
                    for lt in range(4):
                        for n in range(2):
                            op = ops.tile([128, 512], F32, tag="op")
                            for h in range(NHG):
                                mmr(op, hT[:, h, lt * 128:(lt + 1) * 128],
                                    wo_sb[:, h, n * 512:(n + 1) * 512],
                                    start=(h == 0), stop=(h == NHG - 1))
                            ob = ow.tile([128, 512], F32, tag="ob")
                            nc.vector.tensor_tensor(ob, op, bo_sb[:, n * 512:(n + 1) * 512], ADD)
                            nc.sync.dma_start(
                                out=cc_in[lt * 128:(lt + 1) * 128, n * 512:(n + 1) * 512],
                                in_=ob)

            # ======================================================= phase 4
            if OPTS["no_cc"]:
                nc.sync.dma_start(out=out[:, :], in_=cc_in[:, :])
            else:
                nc.gpsimd.collective_compute(
                    "AllReduce", mybir.AluOpType.add,
                    ins=[cc_in[:, :]], outs=[cc_out[:, :]],
                    replica_groups=[[0, 1], [2, 3], [4, 5], [6, 7]],
                )
                nc.sync.dma_start(out=out[:, :], in_=cc_out[:, :])

    return nc


def get_nc():
    key = tuple(sorted(OPTS.items()))
    if key not in _NC_CACHE:
        nc = _build_nc()
        if not nc.is_finalized():
            nc.finalize()
        _NC_CACHE[key] = nc
    return _NC_CACHE[key]


# ---------------------------------------------------------------- host side
def make_in_maps(latents, target, target_mask, target_timestamp,
                 Wq, bq, Wk, bk, Wv, bv, Wo, bo):
    cos_tab, sin_tab = _host_tables()
    P2 = _rot_perm2()

    lat_ts = (np.arange(L, dtype=np.float32) * (MAX_POS - 1) / (L - 1)).astype(np.int64)
    cosq = np.tile(cos_tab[lat_ts].T, (2, 1)).astype(_BF)   # [128, L]
    sinq = np.tile(sin_tab[lat_ts].T, (2, 1)).astype(_BF)

    tabcs = np.ascontiguousarray(np.concatenate(
        [cos_tab, cos_tab, sin_tab, sin_tab], axis=1)).astype(_BF)  # [4096, 256]

    WoT = np.ascontiguousarray(Wo.T)

    in_maps = []
    for core in range(NCORES):
        b, g = core // 2, core % 2
        sl = slice(g * C, (g + 1) * C)
        ts = np.asarray(target_timestamp[b]).astype(np.int64)
        idx_w = np.ascontiguousarray(ts.reshape(T // 128, 128).T.astype(np.int32))
        mask = np.asarray(target_mask[b]).astype(np.uint8)
        m = {
            "tgtT": np.ascontiguousarray(np.asarray(target[b]).T).astype(_BF),
            "latT": np.ascontiguousarray(np.asarray(latents[b]).T).astype(_BF),
            "wkT": np.ascontiguousarray(np.asarray(Wk)[sl, :].T).astype(_BF),
            "wvT": np.ascontiguousarray(np.asarray(Wv)[sl, :].T).astype(_BF),
            "wqT": np.ascontiguousarray(np.asarray(Wq)[sl, :].T).astype(_BF),
            "woT": np.ascontiguousarray(WoT[sl, :]).astype(_BF),
            "pt2": np.ascontiguousarray(P2.T).astype(_BF),
            "cosq": cosq, "sinq": sinq,
            "tabcs": tabcs,
            "idx32": idx_w,
            "eye": np.eye(128, dtype=_BF),
            "maskw": np.ascontiguousarray(mask.reshape(T // 128, 128).T),
            "bkw": np.ascontiguousarray(
                np.asarray(bk)[sl].reshape(C // 128, 128).T.astype(np.float32)),
            "bqw": np.ascontiguousarray(
                np.asarray(bq)[sl].reshape(C // 128, 128).T.astype(np.float32)),
            "bvrep": np.ascontiguousarray(
                np.tile(np.asarray(bv)[sl][None, :], (128, 1)).astype(np.float32)),
            "borep": np.ascontiguousarray(
                np.tile(0.5 * np.asarray(bo)[None, :], (128, 1)).astype(np.float32)),
        }
        if OPTS["no_gather"]:
            m["coskg"] = np.ascontiguousarray(tabcs[ts, 0:128].T)
            m["sinkg"] = np.ascontiguousarray(tabcs[ts, 128:256].T)
        in_maps.append(m)
    return in_maps


def kernel(latents, target, target_mask, target_timestamp,
           Wq, bq, Wk, bk, Wv, bv, Wo, bo, _trace=False, _trace_kwargs=None):
    in_maps = make_in_maps(latents, target, target_mask, target_timestamp,
                           Wq, bq, Wk, bk, Wv, bv, Wo, bo)
    nc = get_nc()
    res = run_bass_kernel_spmd(nc, in_maps, list(range(NCORES)),
                               trace=_trace, **(_trace_kwargs or {}))
    full = np.zeros((B, L, HID), np.float32)
    for b in range(B):
        if OPTS["no_cc"]:
            full[b] = res.results[2 * b]["out"] + res.results[2 * b + 1]["out"]
        else:
            full[b] = res.results[2 * b]["out"]
    if _trace:
        return full, res
    return full


# revision 21
# speedup vs baseline: 1.1331x; 1.1331x over previous
"""Trainium2 Bass kernel for nn_NeuralAttention (cross-attention with RoPE).

Sharding: 8 cores = 4 batches (data parallel) x 2 head-groups (tensor
parallel, 8 heads each).  AllReduce over TP pairs after the output
projection.

Per-core device program (SPMD, per-core data):
  phase 1: Q/K/V projections in transposed layout (channels on partitions),
           RoPE applied via a block rotate-half permutation matmul + DVE
           combine with gathered cos/sin tables (dma_gather transpose).
  phase 2: per head-pair: row-packed score matmuls (d=64 contraction),
           Exp on ScalarE with fused 1/8 scale + per-key mask bias,
           attn@V matmuls with an appended ones column producing the
           softmax denominator for free, then normalization.
  phase 3: output projection (K=64 per head) + bo/2, DMA to DRAM.
  phase 4: AllReduce over {2b, 2b+1} pairs.
"""

import numpy as np
import ml_dtypes

import concourse.bass as bass
import concourse.mybir as mybir
from concourse import bacc
import concourse.tile as tile
from concourse import library_config
from concourse.bass_utils import run_bass_kernel_spmd

B, L, T = 4, 512, 4096
HID, NH, HD = 1024, 16, 64
MAX_POS, BASE = 4096, 10000.0
G = 2                 # TP head groups
NHG = NH // G         # heads per group
C = NHG * HD          # channels per group = 512
NCORES = 8

F32 = mybir.dt.float32
F32R = mybir.dt.float32r
BF16 = mybir.dt.bfloat16
I16 = mybir.dt.int16
U8 = mybir.dt.uint8

MULT = None  # set after import
ADD = None

_BF = ml_dtypes.bfloat16


# ---------------------------------------------------------------- host prep
def _host_tables():
    inv_freq = 1.0 / BASE ** (np.arange(0, HD, 2, dtype=np.float32) / HD)
    t = np.arange(MAX_POS, dtype=np.float32)
    freqs = np.einsum('i,j->ij', t, inv_freq).astype(np.float32)
    emb = np.concatenate([freqs, freqs], axis=-1)          # [MAX_POS, HD]
    return np.cos(emb).astype(np.float32), np.sin(emb).astype(np.float32)


def _rot_perm2():
    # P: rotate_half as a linear map; P2 = blockdiag(P, P)  [128, 128]
    P = np.zeros((HD, HD), np.float32)
    for d in range(HD // 2):
        P[d, d + HD // 2] = -1.0
        P[d + HD // 2, d] = 1.0
    P2 = np.zeros((128, 128), np.float32)
    P2[:64, :64] = P
    P2[64:, 64:] = P
    return P2


# ---------------------------------------------------------------- bass build
_NC_CACHE = {}
OPTS = {"no_cc": False, "no_gather": False}


def _build_nc():
    global MULT, ADD
    MULT = mybir.AluOpType.mult
    ADD = mybir.AluOpType.add
    EXP = mybir.ActivationFunctionType.Exp

    nc = bacc.Bacc(None, target_bir_lowering=False)

    # -------- DRAM parameters (per-core data fed via in_maps)
    tgtT = nc.declare_dram_parameter("tgtT", [HID, T], BF16, isOutput=False)       # target[b].T
    latT = nc.declare_dram_parameter("latT", [HID, L], BF16, isOutput=False)       # latents[b].T
    wkT = nc.declare_dram_parameter("wkT", [HID, C], BF16, isOutput=False)         # Wk_g.T
    wvT = nc.declare_dram_parameter("wvT", [HID, C], BF16, isOutput=False)
    wqT = nc.declare_dram_parameter("wqT", [HID, C], BF16, isOutput=False)
    woT = nc.declare_dram_parameter("woT", [C, HID], BF16, isOutput=False)         # Wo.T rows of group
    pt2 = nc.declare_dram_parameter("pt2", [128, 128], BF16, isOutput=False)       # P2.T
    cosq = nc.declare_dram_parameter("cosq", [128, L], BF16, isOutput=False)       # replicated x2
    sinq = nc.declare_dram_parameter("sinq", [128, L], BF16, isOutput=False)
    tabcs = nc.declare_dram_parameter("tabcs", [MAX_POS, 256], BF16, isOutput=False)  # [cos|cos|sin|sin]
    idx32 = nc.declare_dram_parameter("idx32", [128, T // 128], mybir.dt.int32, isOutput=False)
    eye = nc.declare_dram_parameter("eye", [128, 128], BF16, isOutput=False)
    if OPTS["no_gather"]:
        coskg = nc.declare_dram_parameter("coskg", [128, T], BF16, isOutput=False)
        sinkg = nc.declare_dram_parameter("sinkg", [128, T], BF16, isOutput=False)
    maskw = nc.declare_dram_parameter("maskw", [128, T // 128], U8, isOutput=False)
    bkw = nc.declare_dram_parameter("bkw", [128, C // 128], F32, isOutput=False)
    bqw = nc.declare_dram_parameter("bqw", [128, C // 128], F32, isOutput=False)
    bvrep = nc.declare_dram_parameter("bvrep", [128, C], F32, isOutput=False)
    borep = nc.declare_dram_parameter("borep", [128, HID], F32, isOutput=False)   # bo/2 replicated

    out = nc.declare_dram_parameter("out", [L, HID], F32, isOutput=True)
    cc_in = nc.dram_tensor("cc_in", [L, HID], F32)
    cc_out = nc.dram_tensor("cc_out", [L, HID], F32)

    TS = 512                    # t-slice width for phase 1
    NSL = T // TS               # 16 slices

    def mmr(out_ap, lhsT, rhs, **kw):
        nc.tensor.matmul(out_ap, lhsT, rhs, **kw)

    with tile.TileContext(nc) as tc:
        with tc.tile_pool(name="persist", bufs=1) as persist:
            # persistent across phases
            kpr = [persist.tile([128, T], BF16, tag=f"kpr{i}", name=f"kpr{i}")
                   for i in range(4)]
            qpr = [persist.tile([128, L], BF16, tag=f"qpr{i}", name=f"qpr{i}")
                   for i in range(4)]
            v_sb = persist.tile([128, T // 128, NHG, HD + 1], BF16, tag="v_sb")
            hT = persist.tile([64, NHG, L], BF16, tag="hT")
            ones_sb = persist.tile([128, 64], F32, tag="ones")
            mb_sb = persist.tile([128, T // 128], F32, tag="mb")

            nc.vector.memset(ones_sb[64:65, :], 1.0)
            # ones column of v (per head)
            nc.vector.memset(v_sb[:, :, :, HD:HD + 1], 1.0)

            # mask -> additive bias ( (m-1)*30000 : 0 keep, -30000 drop )
            with tc.tile_pool(name="mprep", bufs=1) as mprep:
                mk_sb = mprep.tile([128, T // 128], U8, tag="mk")
                nc.sync.dma_start(out=mk_sb, in_=maskw[:, :])
                nc.vector.tensor_copy(out=mb_sb, in_=mk_sb)       # u8 -> f32
                nc.vector.tensor_scalar_add(mb_sb, mb_sb, -1.0)
                nc.vector.tensor_scalar_mul(mb_sb, mb_sb, 30000.0)

            pt2_sb = persist.tile([128, 128], BF16, tag="pt2")
            nc.sync.dma_start(out=pt2_sb, in_=pt2[:, :])

            # ======================================================= phase 1
            with tc.tile_pool(name="tgtp", bufs=2) as tgtp, \
                 tc.tile_pool(name="scr", bufs=3) as scr, \
                 tc.tile_pool(name="kps", bufs=2, space="PSUM") as kps, \
                 tc.tile_pool(name="vps", bufs=2, space="PSUM") as vps, \
                 tc.tile_pool(name="rps", bufs=2, space="PSUM") as rps:

                # ---- Q projection + rope (scoped: frees wq/lat right after)
                with tc.tile_pool(name="qc", bufs=1) as qc:
                    wq_sb = qc.tile([128, 8, C], BF16, tag="wq")
                    nc.sync.dma_start(out=wq_sb, in_=wqT[:, :].rearrange("(k p) c -> p k c", p=128))
                    lat_sb = qc.tile([128, 8, L], BF16, tag="lat")
                    nc.sync.dma_start(out=lat_sb, in_=latT[:, :].rearrange("(k p) l -> p k l", p=128))
                    bq_sb = qc.tile([128, C // 128], F32, tag="bq")
                    nc.sync.dma_start(out=bq_sb, in_=bqw[:, :])
                    cq_sb = qc.tile([128, L], BF16, tag="cq")
                    nc.sync.dma_start(out=cq_sb, in_=cosq[:, :])
                    sq_sb = qc.tile([128, L], BF16, tag="sq")
                    nc.sync.dma_start(out=sq_sb, in_=sinq[:, :])

                    for ct in range(4):
                        qp = kps.tile([128, L], F32, tag="kp")
                        for k in range(8):
                            mmr(qp, wq_sb[:, k, ct * 128:(ct + 1) * 128],
                                lat_sb[:, k, :], start=(k == 0), stop=(k == 7))
                        qsb = scr.tile([128, L], BF16, tag="ksb")
                        nc.vector.tensor_scalar_add(qsb, qp, bq_sb[:, ct:ct + 1])
                        qr = rps.tile([128, L], F32, tag="rp")
                        mmr(qr, pt2_sb, qsb, start=True, stop=True)
                        t1 = scr.tile([128, L], BF16, tag="t1")
                        nc.vector.tensor_tensor(t1, qsb, cq_sb, MULT)
                        t2 = scr.tile([128, L], BF16, tag="t2")
                        nc.vector.tensor_tensor(t2, qr, sq_sb, MULT)
                        nc.vector.tensor_tensor(qpr[ct], t1, t2, ADD)

                # ---- K/V weights + rope tables (after Q scope frees space)
                ph1c_cm = tc.tile_pool(name="ph1c", bufs=1)
                ph1c = ph1c_cm.__enter__()
                wk_sb = ph1c.tile([128, 8, C], BF16, tag="wk")
                nc.sync.dma_start(out=wk_sb, in_=wkT[:, :].rearrange("(k p) c -> p k c", p=128))
                wv_sb = ph1c.tile([128, 8, C], BF16, tag="wv")
                nc.sync.dma_start(out=wv_sb, in_=wvT[:, :].rearrange("(k p) c -> p k c", p=128))
                bk_sb = ph1c.tile([128, C // 128], F32, tag="bk")
                nc.sync.dma_start(out=bk_sb, in_=bkw[:, :])
                bv_sb = ph1c.tile([128, C], F32, tag="bv")
                nc.sync.dma_start(out=bv_sb, in_=bvrep[:, :])

                # gathered + replicated rope tables (bf16), [128, T]
                cosk_sb = ph1c.tile([128, 1, T], BF16, tag="cosk")
                sink_sb = ph1c.tile([128, 1, T], BF16, tag="sink")
                if OPTS["no_gather"]:
                    nc.sync.dma_start(out=cosk_sb[:, 0, :], in_=coskg[:, :])
                    nc.sync.dma_start(out=sink_sb[:, 0, :], in_=sinkg[:, :])
                else:
                    # gather [cos|cos|sin|sin] rows by timestamp (one row per
                    # partition per call), then PE-transpose into [chan, t]
                    idx_sb = ph1c.tile([128, T // 128], mybir.dt.int32, tag="idx")
                    nc.sync.dma_start(out=idx_sb, in_=idx32[:, :])
                    eye_sb = ph1c.tile([128, 128], BF16, tag="eye")
                    nc.sync.dma_start(out=eye_sb, in_=eye[:, :])
                    tcs_sb = ph1c.tile([128, T // 128, 256], BF16, tag="tcs")
                    with tc.tile_pool(name="tps", bufs=2, space="PSUM") as tps:
                        for tt in range(T // 128):
                            nc.gpsimd.indirect_dma_start(
                                out=tcs_sb[:, tt, :], out_offset=None,
                                in_=tabcs[:, :],
                                in_offset=bass.IndirectOffsetOnAxis(
                                    ap=idx_sb[:, tt:tt + 1], axis=0))
                        for tt in range(T // 128):
                            tpc = tps.tile([128, 128], BF16, tag="tp")
                            nc.tensor.transpose(out=tpc, in_=tcs_sb[:, tt, 0:128],
                                                identity=eye_sb)
                            nc.vector.tensor_copy(
                                out=cosk_sb[:, 0, tt * 128:(tt + 1) * 128], in_=tpc)
                            tpsn = tps.tile([128, 128], BF16, tag="tp")
                            nc.tensor.transpose(out=tpsn, in_=tcs_sb[:, tt, 128:256],
                                                identity=eye_sb)
                            nc.vector.tensor_copy(
                                out=sink_sb[:, 0, tt * 128:(tt + 1) * 128], in_=tpsn)

                # ---- K/V projections + K rope, streamed over t-slices
                for s in range(NSL):
                    tg = tgtp.tile([128, 8, TS], BF16, tag="tgt")
                    nc.sync.dma_start(
                        out=tg,
                        in_=tgtT[:, s * TS:(s + 1) * TS].rearrange("(k p) t -> p k t", p=128))
                    cosf = cosk_sb[:, 0, s * TS:(s + 1) * TS]
                    sinf = sink_sb[:, 0, s * TS:(s + 1) * TS]

                    ksbs = []
                    for ct in range(4):
                        kp = kps.tile([128, TS], F32, tag="kp")
                        for k in range(8):
                            mmr(kp, wk_sb[:, k, ct * 128:(ct + 1) * 128],
                                tg[:, k, :], start=(k == 0), stop=(k == 7))
                        ksb = scr.tile([128, TS], BF16, tag="ksb")
                        nc.vector.tensor_scalar_add(ksb, kp, bk_sb[:, ct:ct + 1])
                        ksbs.append(ksb)
                    # V while rope matmuls depend on the copies above
                    for tt in range(TS // 128):
                        vp = vps.tile([128, C], F32, tag="vp")
                        for k in range(8):
                            mmr(vp, tg[:, k, tt * 128:(tt + 1) * 128],
                                wv_sb[:, k, :], start=(k == 0), stop=(k == 7))
                        ti = s * (TS // 128) + tt
                        nc.vector.tensor_tensor(
                            v_sb[:, ti, :, 0:HD],
                            vp.rearrange("p (h d) -> p h d", h=NHG),
                            bv_sb.rearrange("p (h d) -> p h d", h=NHG), ADD)
                    for ct in range(4):
                        kr = rps.tile([128, TS], F32, tag="rp")
                        mmr(kr, pt2_sb, ksbs[ct], start=True, stop=True)
                        t1 = scr.tile([128, TS], BF16, tag="t1")
                        nc.vector.tensor_tensor(t1, ksbs[ct], cosf, MULT)
                        t2 = scr.tile([128, TS], BF16, tag="t2")
                        nc.vector.tensor_tensor(t2, kr, sinf, MULT)
                        nc.vector.tensor_tensor(
                            kpr[ct][:, s * TS:(s + 1) * TS], t1, t2, ADD)
                ph1c_cm.__exit__(None, None, None)

            # ======================================================= phase 2
            with tc.tile_pool(name="ph2c", bufs=1) as ph2c, \
                 tc.tile_pool(name="escr", bufs=4) as escr, \
                 tc.tile_pool(name="scr2", bufs=2) as scr2:

                wo_sb = ph2c.tile([64, NHG, HID], BF16, tag="wo")
                nc.sync.dma_start(out=wo_sb, in_=woT[:, :].rearrange("(h p) o -> p h o", p=64))
                bo_sb = ph2c.tile([128, HID], F32, tag="bo")
                nc.sync.dma_start(out=bo_sb, in_=borep[:, :])

                ph2p_cm = tc.tile_pool(name="sps", bufs=2, space="PSUM")
                sps = ph2p_cm.__enter__()
                avp_cm = tc.tile_pool(name="avp", bufs=1, space="PSUM")
                avp = avp_cm.__enter__()
                bcp_cm = tc.tile_pool(name="bcp", bufs=2, space="PSUM")
                bcp = bcp_cm.__enter__()

                NT = T // 128     # 32 key tiles
                for p in range(4):
                    hA, hB = 2 * p, 2 * p + 1
                    avA = avp.tile([65, L], F32, tag="avA")
                    avB = avp.tile([65, L], F32, tag="avB")
                    es = {}
                    for tt in range(NT):
                        sAB = sps.tile([128, 2, L], F32, tag="sAB")
                        nc.tensor.matmul(sAB[:, 0, :],
                                         kpr[p][0:64, tt * 128:(tt + 1) * 128],
                                         qpr[p][0:64, :], start=True, stop=True)
                        nc.tensor.matmul(sAB[:, 1, :],
                                         kpr[p][64:128, tt * 128:(tt + 1) * 128],
                                         qpr[p][64:128, :], start=True, stop=True)
                        eAB = escr.tile([128, 2, L], BF16, tag="eAB")
                        nc.scalar.activation(out=eAB, in_=sAB, func=EXP,
                                             bias=mb_sb[:, tt:tt + 1], scale=0.125)
                        es[tt] = eAB
                        # software-pipeline: issue previous tile's AV matmuls
                        if tt > 0:
                            eP = es.pop(tt - 1)
                            nc.tensor.matmul(avA, v_sb[:, tt - 1, hA, :], eP[:, 0, :],
                                             start=(tt - 1 == 0), stop=False)
                            nc.tensor.matmul(avB, v_sb[:, tt - 1, hB, :], eP[:, 1, :],
                                             start=(tt - 1 == 0), stop=False)
                    eP = es.pop(NT - 1)
                    nc.tensor.matmul(avA, v_sb[:, NT - 1, hA, :], eP[:, 0, :],
                                     start=False, stop=True)
                    nc.tensor.matmul(avB, v_sb[:, NT - 1, hB, :], eP[:, 1, :],
                                     start=False, stop=True)

                    for av, h in ((avA, hA), (avB, hB)):
                        dn = scr2.tile([128, L], F32, tag="dn")
                        nc.vector.tensor_copy(out=dn[64:65, :], in_=av[64:65, :])
                        nc.vector.reciprocal(out=dn[64:65, :], in_=dn[64:65, :])
                        bc = bcp.tile([64, L], F32, tag="bc")
                        nc.tensor.matmul(bc, ones_sb[64:65, :], dn[64:65, :], start=True, stop=True)
                        osb = scr2.tile([64, L], F32, tag="osb")
                        nc.vector.tensor_copy(out=osb, in_=av[0:64, :])
                        nc.vector.tensor_tensor(hT[:, h, :], osb, bc, MULT)

                bcp_cm.__exit__(None, None, None)
                avp_cm.__exit__(None, None, None)
                ph2p_cm.__exit__(None, None, None)

                # ============================== phase 3 + overlapped reduce
                with tc.tile_pool(name="ops", bufs=2, space="PSUM") as ops, \
                     tc.tile_pool(name="ow", bufs=3) as ow:
                    for half in range(2):
                        for lt in range(2 * half, 2 * half + 2):
                            for n in range(2):
                                op = ops.tile([128, 512], F32, tag="op")
                                for h in range(NHG):
                                    mmr(op, hT[:, h, lt * 128:(lt + 1) * 128],
                                        wo_sb[:, h, n * 512:(n + 1) * 512],
                                        start=(h == 0), stop=(h == NHG - 1))
                                ob = ow.tile([128, 512], F32, tag="ob")
                                nc.vector.tensor_tensor(
                                    ob, op, bo_sb[:, n * 512:(n + 1) * 512], ADD)
                                nc.sync.dma_start(
                                    out=cc_in[lt * 128:(lt + 1) * 128,
                                              n * 512:(n + 1) * 512],
                                    in_=ob)
                        hs = slice(half * (L // 2), (half + 1) * (L // 2))
                        if OPTS["no_cc"]:
                            nc.sync.dma_start(out=out[hs, :], in_=cc_in[hs, :])
                        else:
                            nc.gpsimd.collective_compute(
                                "AllReduce", mybir.AluOpType.add,
                                ins=[cc_in[hs, :]], outs=[cc_out[hs, :]],
                                replica_groups=[[0, 1], [2, 3], [4, 5], [6, 7]],
                            )
                            nc.sync.dma_start(out=out[hs, :], in_=cc_out[hs, :])

    return nc


def get_nc():
    key = tuple(sorted(OPTS.items()))
    if key not in _NC_CACHE:
        nc = _build_nc()
        if not nc.is_finalized():
            nc.finalize()
        _NC_CACHE[key] = nc
    return _NC_CACHE[key]


# ---------------------------------------------------------------- host side
def make_in_maps(latents, target, target_mask, target_timestamp,
                 Wq, bq, Wk, bk, Wv, bv, Wo, bo):
    cos_tab, sin_tab = _host_tables()
    P2 = _rot_perm2()

    lat_ts = (np.arange(L, dtype=np.float32) * (MAX_POS - 1) / (L - 1)).astype(np.int64)
    cosq = np.tile(cos_tab[lat_ts].T, (2, 1)).astype(_BF)   # [128, L]
    sinq = np.tile(sin_tab[lat_ts].T, (2, 1)).astype(_BF)

    tabcs = np.ascontiguousarray(np.concatenate(
        [cos_tab, cos_tab, sin_tab, sin_tab], axis=1)).astype(_BF)  # [4096, 256]

    WoT = np.ascontiguousarray(Wo.T)

    in_maps = []
    for core in range(NCORES):
        b, g = core // 2, core % 2
        sl = slice(g * C, (g + 1) * C)
        ts = np.asarray(target_timestamp[b]).astype(np.int64)
        idx_w = np.ascontiguousarray(ts.reshape(T // 128, 128).T.astype(np.int32))
        mask = np.asarray(target_mask[b]).astype(np.uint8)
        m = {
            "tgtT": np.ascontiguousarray(np.asarray(target[b]).T).astype(_BF),
            "latT": np.ascontiguousarray(np.asarray(latents[b]).T).astype(_BF),
            "wkT": np.ascontiguousarray(np.asarray(Wk)[sl, :].T).astype(_BF),
            "wvT": np.ascontiguousarray(np.asarray(Wv)[sl, :].T).astype(_BF),
            "wqT": np.ascontiguousarray(np.asarray(Wq)[sl, :].T).astype(_BF),
            "woT": np.ascontiguousarray(WoT[sl, :]).astype(_BF),
            "pt2": np.ascontiguousarray(P2.T).astype(_BF),
            "cosq": cosq, "sinq": sinq,
            "tabcs": tabcs,
            "idx32": idx_w,
            "eye": np.eye(128, dtype=_BF),
            "maskw": np.ascontiguousarray(mask.reshape(T // 128, 128).T),
            "bkw": np.ascontiguousarray(
                np.asarray(bk)[sl].reshape(C // 128, 128).T.astype(np.float32)),
            "bqw": np.ascontiguousarray(
                np.asarray(bq)[sl].reshape(C // 128, 128).T.astype(np.float32)),
            "bvrep": np.ascontiguousarray(
                np.tile(np.asarray(bv)[sl][None, :], (128, 1)).astype(np.float32)),
            "borep": np.ascontiguousarray(
                np.tile(0.5 * np.asarray(bo)[None, :], (128, 1)).astype(np.float32)),
        }
        if OPTS["no_gather"]:
            m["coskg"] = np.ascontiguousarray(tabcs[ts, 0:128].T)
            m["sinkg"] = np.ascontiguousarray(tabcs[ts, 128:256].T)
        in_maps.append(m)
    return in_maps


def kernel(latents, target, target_mask, target_timestamp,
           Wq, bq, Wk, bk, Wv, bv, Wo, bo, _trace=False, _trace_kwargs=None):
    in_maps = make_in_maps(latents, target, target_mask, target_timestamp,
                           Wq, bq, Wk, bk, Wv, bv, Wo, bo)
    nc = get_nc()
    res = run_bass_kernel_spmd(nc, in_maps, list(range(NCORES)),
                               trace=_trace, **(_trace_kwargs or {}))
    full = np.zeros((B, L, HID), np.float32)
    for b in range(B):
        if OPTS["no_cc"]:
            full[b] = res.results[2 * b]["out"] + res.results[2 * b + 1]["out"]
        else:
            full[b] = res.results[2 * b]["out"]
    if _trace:
        return full, res
    return full
